# revision 4
# baseline (speedup 1.0000x reference)
"""Multi-head attention (b=2, n=2048, dim=1024, 16 heads x 64) on 8 TRN2 NeuronCores.

Sharding: core c handles batch c//4 and heads 4*(c%4) .. 4*(c%4)+3
(data parallel over batch x 4-way head/tensor parallel). w_qkv is
column-sharded by head; w_out is column-sharded: each core computes a
256-column slice of the output after AllGathers of the attention outputs
within its 4-core batch group (no all-reduce needed).

Device layout is feature-major ("K-major"): x arrives pre-transposed
[dim, n] in bf16; Q^T/K^T are produced feature-major (weight-stationary
matmul order to minimize LDWEIGHTS) and V token-major; attention scores
are computed transposed (dotsT[k, q]); softmax sums come from an
augmented ones-column in the V matmul; softmax exp runs on the scalar
engine with the 1/sqrt(d) scale folded in. 1/Z uses the fast custom-DVE
reciprocal; Z^-1 is broadcast across partitions by a rank-1 PE matmul.
The AllGather is split per (head-pair, 512-token quarter) — 8 small
collectives fired as each attention block finishes — and the output
projection for each gathered piece is interleaved into the ACT-bound
attention steady state so only the last piece remains after attention.
The final output is produced transposed [cols, n]; the host transposes
back.
"""

import sys

sys.path.insert(0, "/opt/trn_rl_repo")

import ml_dtypes
import numpy as np

import concourse.bass as bass  # noqa: F401  (engine types)
import concourse.tile as tile
from concourse import bacc, mybir
from concourse.bass_utils import run_bass_kernel_spmd

F32 = mybir.dt.float32
F32R = mybir.dt.float32r
BF16 = mybir.dt.bfloat16
NP_BF16 = np.dtype(ml_dtypes.bfloat16)

# Problem constants
B, N, DIM = 2, 2048, 1024
HEADS, DH = 16, 64
INNER = HEADS * DH
SCALE = DH ** -0.5
CORES = 8
GROUP_SIZE = 4
REPLICA_GROUPS = [[0, 1, 2, 3], [4, 5, 6, 7]]
HPC = 4  # heads per core
CS = HPC * DH  # 256 per-core feature columns

KC = DIM // 128  # 8 contraction chunks for dim
TT = N // 128  # 16 token tiles
QB = N // 512  # 4 q blocks
NKC = N // 128  # 16 key chunks
NBLK = 2 * QB  # 8 attention blocks: (head pair, 512-query quarter)


def build_nc():
    nc = bacc.Bacc("TRN2", target_bir_lowering=False, debug=False, num_devices=CORES)
    xt = nc.dram_tensor("xt", [DIM, N], BF16, kind="ExternalInput").ap()
    wq = nc.dram_tensor("wq", [DIM, CS], BF16, kind="ExternalInput").ap()
    wk = nc.dram_tensor("wk", [DIM, CS], BF16, kind="ExternalInput").ap()
    wv = nc.dram_tensor("wv", [DIM, CS], BF16, kind="ExternalInput").ap()
    wo = nc.dram_tensor("wo", [INNER, CS], BF16, kind="ExternalInput").ap()
    bo = nc.dram_tensor("bo", [CS], F32, kind="ExternalInput").ap()
    y = nc.dram_tensor("y", [CS, N], F32, kind="ExternalOutput").ap()  # y^T

    cc_in = [nc.dram_tensor(f"cc_in{b}", [128, 512], BF16) for b in range(NBLK)]
    cc_out = [
        nc.dram_tensor(f"cc_out{b}", [GROUP_SIZE * 128, 512], BF16)
        for b in range(NBLK)
    ]

    with tile.TileContext(nc) as tc:
        with (
            tc.tile_pool(name="big", bufs=2) as big,  # xt, then the AG results
            tc.tile_pool(name="sb", bufs=1) as sb,
            tc.tile_pool(name="expp", bufs=4) as expp,
            tc.tile_pool(name="yout", bufs=3) as yout,
            tc.tile_pool(name="norm", bufs=4) as normp,
            tc.tile_pool(name="zv", bufs=4) as zvp,
            tc.tile_pool(name="psd", bufs=2, space="PSUM") as psd,  # 4 banks
            tc.tile_pool(name="pso", bufs=2, space="PSUM") as pso,  # 2 banks
            tc.tile_pool(name="pzb", bufs=1, space="PSUM") as pzb,  # 1 bank
            tc.tile_pool(name="psy", bufs=1, space="PSUM") as psyp,  # 1 bank
        ):
            # ---- load inputs -------------------------------------------------
            xt_sb = big.tile([128, KC, N], BF16, tag="bigbuf")
            wq_sb = sb.tile([128, KC, CS], BF16)
            wk_sb = sb.tile([128, KC, CS], BF16)
            wv_sb = sb.tile([128, KC, CS], BF16)
            wo_sb = sb.tile([128, KC, CS], BF16)
            nc.sync.dma_start(out=wq_sb, in_=wq.rearrange("(c p) n -> p c n", p=128))
            nc.sync.dma_start(out=wk_sb, in_=wk.rearrange("(c p) n -> p c n", p=128))
            xt_r = xt.rearrange("(c p) n -> p c n", p=128)
            for c in range(KC):
                for qb in range(QB):
                    sl = slice(qb * 512, (qb + 1) * 512)
                    nc.sync.dma_start(out=xt_sb[:, c, sl], in_=xt_r[:, c, sl])
            nc.sync.dma_start(out=wv_sb, in_=wv.rearrange("(c p) n -> p c n", p=128))
            nc.sync.dma_start(out=wo_sb, in_=wo.rearrange("(c p) n -> p c n", p=128))

            # bias, transposed layout: partition = column-within-block
            bias_sb = sb.tile([128, 2], F32)
            nc.sync.dma_start(out=bias_sb, in_=bo.rearrange("(cb p) -> p cb", p=128))

            ones_f = sb.tile([128, TT], F32)
            nc.vector.memset(ones_f, 1.0)
            ones_r = sb.tile([1, DH], F32R)
            nc.vector.tensor_copy(ones_r, ones_f[0:1, 0:1].broadcast_to([1, DH]))

            # ---- QKV projection ---------------------------------------------
            qt_sb = sb.tile([128, 2, N], BF16)
            kt_sb = sb.tile([128, 2, N], BF16)
            vaug = sb.tile([128, TT, HPC, DH + 1], BF16)
            with nc.allow_low_precision(reason="bf16 ones column"):
                for h in range(HPC):
                    nc.vector.tensor_copy(vaug[:, :, h, DH], ones_f)

            def qk_pass(m, dst, w_sb):
                # weight-stationary: one LDWEIGHTS per (m, c); 4 query blocks
                # stream through the same loaded weights.
                pa = psd.tile([128, 2, 512], F32, name="psd")
                pb = psd.tile([128, 2, 512], F32, name="psd")
                accs = [pa[:, 0, :], pa[:, 1, :], pb[:, 0, :], pb[:, 1, :]]
                for c in range(KC):
                    for qb in range(QB):
                        nc.tensor.matmul(
                            accs[qb],
                            lhsT=w_sb[:, c, m * 128 : (m + 1) * 128],
                            rhs=xt_sb[:, c, qb * 512 : (qb + 1) * 512],
                            start=(c == 0),
                            stop=(c == KC - 1),
                        )
                with nc.allow_low_precision(reason="bf16 attention"):
                    for qb in range(QB):
                        nc.vector.tensor_copy(
                            dst[:, m, qb * 512 : (qb + 1) * 512], accs[qb]
                        )

            for m in range(2):
                qk_pass(m, qt_sb, wq_sb)
                qk_pass(m, kt_sb, wk_sb)
            for t in range(TT):
                ps = psd.tile([128, 2, 512], F32, name="psd")
                acc = ps[:, 0, 0:CS]
                for c in range(KC):
                    nc.tensor.matmul(
                        acc,
                        lhsT=xt_sb[:, c, t * 128 : (t + 1) * 128],
                        rhs=wv_sb[:, c, :],
                        start=(c == 0),
                        stop=(c == KC - 1),
                    )
                with nc.allow_low_precision(reason="bf16 attention"):
                    nc.vector.tensor_copy(
                        vaug[:, t, :, 0:DH],
                        acc.rearrange("p (h d) -> p h d", d=DH),
                    )

            # ---- attention + per-block AllGather + interleaved out-proj -----
            outt_sb = sb.tile([128, 2, N], BF16)
            y_acc = sb.tile([128, 2, N], F32)
            ag_all = big.tile(
                [128, 2, QB, GROUP_SIZE, 512], BF16, tag="bigbuf"
            )  # [p, hp, qb, src_core, tok]

            def emit_dots(blk, kc):
                hp, qb = divmod(blk, QB)
                ps = psd.tile([128, 2, 512], F32, name="psd")
                for hh in range(2):
                    base = hh * DH
                    nc.tensor.matmul(
                        ps[:, hh, :],
                        lhsT=kt_sb[base : base + DH, hp, kc * 128 : (kc + 1) * 128],
                        rhs=qt_sb[base : base + DH, hp, qb * 512 : (qb + 1) * 512],
                        start=True,
                        stop=True,
                        tile_position=(base, 0),
                    )
                ex = expp.tile([128, 2, 512], BF16, name="expT")
                nc.scalar.activation(
                    out=ex, in_=ps, func=mybir.ActivationFunctionType.Exp, scale=SCALE
                )
                return ex

            def emit_attv(blk, kc, ex, po):
                hp = blk // QB
                for hh in range(2):
                    nc.tensor.matmul(
                        po[hh],
                        lhsT=vaug[:, kc, hp * 2 + hh, :],
                        rhs=ex[:, hh, :],
                        start=(kc == 0),
                        stop=(kc == NKC - 1),
                    )

            def emit_posb(po):
                # drain PSUM accumulators to SBUF right away to free the ring
                po_sbs = []
                for hh in range(2):
                    po_sb = normp.tile([DH + 1, 512], F32, name="po_sb")
                    nc.vector.tensor_copy(po_sb, po[hh])
                    po_sbs.append(po_sb)
                return po_sbs

            def emit_recip(po_sbs):
                zinvs = []
                for hh in range(2):
                    zi = zvp.tile([1, 512], F32, name="zi")
                    nc.vector.reciprocal(out=zi, in_=po_sbs[hh][DH : DH + 1, :])
                    zir = zvp.tile([1, 512], F32R, name="zir")
                    with nc.allow_low_precision(reason="f32r zinv"):
                        nc.vector.tensor_copy(zir, zi)
                    zinvs.append(zir)
                return zinvs

            def emit_zb(zinvs, hh):
                zb = pzb.tile([DH, 512], F32, name="zb")
                nc.tensor.matmul(zb, lhsT=ones_r, rhs=zinvs[hh], start=True, stop=True)
                return zb

            def emit_mul(blk, po_sbs, zb, hh):
                hp, qb = divmod(blk, QB)
                base = hh * DH
                with nc.allow_low_precision(reason="bf16 attention out"):
                    nc.vector.tensor_mul(
                        outt_sb[base : base + DH, hp, qb * 512 : (qb + 1) * 512],
                        po_sbs[hh][0:DH, :],
                        zb,
                    )

            def emit_ag(blk):
                hp, qb = divmod(blk, QB)
                sl = slice(qb * 512, (qb + 1) * 512)
                nc.gpsimd.dma_start(out=cc_in[blk].ap(), in_=outt_sb[:, hp, sl])
                nc.gpsimd.collective_compute(
                    "AllGather",
                    mybir.AluOpType.bypass,
                    ins=[cc_in[blk].ap().opt()],
                    outs=[cc_out[blk].ap().opt()],
                    replica_groups=REPLICA_GROUPS,
                )
                nc.sync.dma_start(
                    out=ag_all[:, hp, qb, :, :],
                    in_=cc_out[blk].ap().rearrange("(c p) n -> p c n", p=128),
                )

            # --- interleaved output projection -------------------------------
            # proj half-piece (hp, qb): y^T[:, qb] (+)= wo[hp]^T @ ag[hp][qb].
            # hp=0 initializes y_acc (bias folded in); hp=1 adds and stores.
            # Each half is 8 matmuls (2 col-blocks x 4 gathered cores) plus 2
            # DVE ops, broken into single-step chunks so it rides the PE slack
            # of the ACT-bound attention loop.
            def proj_tasks(hp, qb):
                tasks = []
                ps_ref = {}

                def mk_mm(cb, c0):
                    def f():
                        if c0 == 0:
                            ps_ref[cb] = psyp.tile([128, 512], F32, name="psy")
                        for c in range(c0, c0 + 2):
                            nc.tensor.matmul(
                                ps_ref[cb],
                                lhsT=wo_sb[:, hp * 4 + c, cb * 128 : (cb + 1) * 128],
                                rhs=ag_all[:, hp, qb, c, :],
                                start=(c == 0),
                                stop=(c == 3),
                            )

                    return f

                def mk_fin(cb):
                    def f():
                        qsl = slice(qb * 512, (qb + 1) * 512)
                        if hp == 0:
                            nc.vector.tensor_scalar_add(
                                out=y_acc[:, cb, qsl],
                                in0=ps_ref[cb],
                                scalar1=bias_sb[:, cb : cb + 1],
                            )
                        else:
                            y_sb = yout.tile([128, 512], F32, name="y_sb")
                            nc.vector.tensor_add(y_sb, ps_ref[cb], y_acc[:, cb, qsl])
                            nc.sync.dma_start(
                                out=y[cb * 128 : (cb + 1) * 128, qsl], in_=y_sb
                            )

                    return f

                for cb in range(2):
                    tasks.append(mk_mm(cb, 0))
                    tasks.append(mk_mm(cb, 2))
                    tasks.append(mk_fin(cb))
                return tasks

            # schedule: during block b, emit proj for pieces whose AG fired
            # ~1.5 blocks earlier. AG(b) fires at (b, kc==7).
            proj_sched = {
                2: [(0, 0)],
                3: [(0, 1)],
                4: [(0, 2)],
                5: [(0, 3)],
                6: [(1, 0)],
                7: [(1, 1), (1, 2)],
            }

            # one continuous software-pipelined stream over all 8 blocks:
            # attV lags dots/exp by one step; po drains to SBUF right after a
            # block's last attV; recip/broadcast/mul stages are spread over
            # the next block's early steps; the AllGather fires at kc==7.
            pend_attv = None  # (blk, kc, ex)
            po_cur = None
            posb_prev = None  # po_sbs of previous block
            zinv_prev = None
            zb_prev = [None, None]
            task_q = []
            for step in range(NBLK * NKC):
                blk, kc = divmod(step, NKC)
                if kc == 0:
                    po_prev = po_cur
                    po_cur = [
                        pso.tile([DH + 1, 512], F32, name="ps_o") for _ in range(2)
                    ]
                    task_q = [
                        t for hq in proj_sched.get(blk, []) for t in proj_tasks(*hq)
                    ]
                ex = emit_dots(blk, kc)
                if pend_attv is not None:
                    pblk, pkc, pex = pend_attv
                    emit_attv(pblk, pkc, pex, po_cur if pblk == blk else po_prev)
                    if pkc == NKC - 1:
                        posb_cur = emit_posb(po_prev)
                pend_attv = (blk, kc, ex)
                if blk > 0:
                    if kc == 0:
                        posb_prev = posb_cur
                    elif kc == 1:
                        zinv_prev = emit_recip(posb_prev)
                    elif kc == 5:
                        zb_prev[0] = emit_zb(zinv_prev, 0)
                    elif kc == 6:
                        emit_mul(blk - 1, posb_prev, zb_prev[0], 0)
                    elif kc == 7:
                        zb_prev[1] = emit_zb(zinv_prev, 1)
                    elif kc == 8:
                        emit_mul(blk - 1, posb_prev, zb_prev[1], 1)
                    elif kc == 9:
                        emit_ag(blk - 1)
                if kc >= 8 and task_q:
                    task_q.pop(0)()
            # drain: finish any leftover proj tasks, then the last block's
            # attV / norm / AG and the final projection piece.
            for t in task_q:
                t()
            pblk, pkc, pex = pend_attv
            emit_attv(pblk, pkc, pex, po_cur)
            po_sbs = emit_posb(po_cur)
            zinvs = emit_recip(po_sbs)
            for hh in range(2):
                zb = emit_zb(zinvs, hh)
                emit_mul(NBLK - 1, po_sbs, zb, hh)
            emit_ag(NBLK - 1)
            for t in proj_tasks(1, QB - 1):
                t()

    nc.compile()
    return nc


_NC_CACHE = None


def _get_nc():
    global _NC_CACHE
    if _NC_CACHE is None:
        _NC_CACHE = build_nc()
    return _NC_CACHE


def _wo_perm(w_out):
    # chunk order [AG-hp0: r0..r3 -> w_out rows 256r..256r+128,
    #              AG-hp1: r0..r3 -> w_out rows 256r+128..256r+256]
    blocks = [w_out[256 * r : 256 * r + 128] for r in range(4)]
    blocks += [w_out[256 * r + 128 : 256 * r + 256] for r in range(4)]
    return np.concatenate(blocks, axis=0)


def _make_in_maps(x, w_qkv, w_out, b_out):
    wop = _wo_perm(w_out)
    in_maps = []
    for c in range(CORES):
        bi = c // GROUP_SIZE
        g = c % GROUP_SIZE
        cols = slice(g * CS, (g + 1) * CS)
        in_maps.append(
            {
                "xt": np.ascontiguousarray(x[bi].T).astype(NP_BF16),
                "wq": np.ascontiguousarray(w_qkv[:, cols]).astype(NP_BF16),
                "wk": np.ascontiguousarray(w_qkv[:, INNER:][:, cols]).astype(NP_BF16),
                "wv": np.ascontiguousarray(w_qkv[:, 2 * INNER:][:, cols]).astype(
                    NP_BF16
                ),
                "wo": np.ascontiguousarray(wop[:, cols]).astype(NP_BF16),
                "bo": np.ascontiguousarray(b_out[cols]),
            }
        )
    return in_maps


def _assemble(results):
    out = np.empty((B, N, DIM), dtype=np.float32)
    for c in range(CORES):
        bi = c // GROUP_SIZE
        g = c % GROUP_SIZE
        out[bi, :, g * CS : (g + 1) * CS] = results[c]["y"].T
    return out


def kernel(x, w_qkv, w_out, b_out, _trace=False, _trace_kwargs=None):
    x = np.asarray(x, dtype=np.float32)
    w_qkv = np.asarray(w_qkv, dtype=np.float32)
    w_out = np.asarray(w_out, dtype=np.float32)
    b_out = np.asarray(b_out, dtype=np.float32)
    nc = _get_nc()
    in_maps = _make_in_maps(x, w_qkv, w_out, b_out)
    res = run_bass_kernel_spmd(
        nc,
        in_maps,
        core_ids=list(range(CORES)),
        trace=_trace,
        **(_trace_kwargs or {}),
    )
    out = _assemble(res.results)
    if _trace:
        return out, res
    return out


# revision 10
# speedup vs baseline: 1.2366x; 1.2366x over previous
"""Multi-head attention (b=2, n=2048, dim=1024, 16 heads x 64) on 8 TRN2 NeuronCores.

Sharding: core c handles batch c//4 and heads 4*(c%4) .. 4*(c%4)+3
(data parallel over batch x 4-way head/tensor parallel). w_qkv is
column-sharded by head; w_out is column-sharded: each core computes a
256-column slice of the output after AllGathers of the attention outputs
within its 4-core batch group (no all-reduce needed).

Device layout is feature-major ("K-major"): x arrives pre-transposed
[dim, n] in bf16; Q^T/K^T are produced feature-major (weight-stationary
matmul order to minimize LDWEIGHTS) and V token-major; attention scores
are computed transposed (dotsT[k, q]); softmax sums come from an
augmented ones-column in the V matmul; softmax exp runs on the scalar
engine with the 1/sqrt(d) scale folded in. 1/Z uses the fast custom-DVE
reciprocal; Z^-1 is broadcast across partitions by a rank-1 PE matmul.
The AllGather is split per (head-pair, 512-token quarter) — 8 small
collectives fired as each attention block finishes — and the output
projection for each gathered piece is interleaved into the ACT-bound
attention steady state so only the last piece remains after attention.
The final output is produced transposed [cols, n]; the host transposes
back.
"""

import sys

sys.path.insert(0, "/opt/trn_rl_repo")

import ml_dtypes
import numpy as np

import concourse.bass as bass  # noqa: F401  (engine types)
import concourse.tile as tile
from concourse import bacc, mybir
from concourse.bass_utils import run_bass_kernel_spmd

F32 = mybir.dt.float32
F32R = mybir.dt.float32r
BF16 = mybir.dt.bfloat16
NP_BF16 = np.dtype(ml_dtypes.bfloat16)

# Problem constants
B, N, DIM = 2, 2048, 1024
HEADS, DH = 16, 64
INNER = HEADS * DH
SCALE = DH ** -0.5
CORES = 8
GROUP_SIZE = 4
REPLICA_GROUPS = [[0, 1, 2, 3], [4, 5, 6, 7]]
HPC = 4  # heads per core
CS = HPC * DH  # 256 per-core feature columns

KC = DIM // 128  # 8 contraction chunks for dim
TT = N // 128  # 16 token tiles
QB = N // 512  # 4 q blocks
NKC = N // 128  # 16 key chunks
NBLK = 2 * QB  # 8 attention blocks: (head pair, 512-query quarter)


def build_nc():
    nc = bacc.Bacc("TRN2", target_bir_lowering=False, debug=False, num_devices=CORES)
    xt = nc.dram_tensor("xt", [DIM, N], BF16, kind="ExternalInput").ap()
    wq = nc.dram_tensor("wq", [DIM, CS], BF16, kind="ExternalInput").ap()
    wk = nc.dram_tensor("wk", [DIM, CS], BF16, kind="ExternalInput").ap()
    wv = nc.dram_tensor("wv", [DIM, CS], BF16, kind="ExternalInput").ap()
    wo = nc.dram_tensor("wo", [INNER, CS], BF16, kind="ExternalInput").ap()
    bo = nc.dram_tensor("bo", [CS], F32, kind="ExternalInput").ap()
    y = nc.dram_tensor("y", [CS, N], F32, kind="ExternalOutput").ap()  # y^T

    cc_in = [nc.dram_tensor(f"cc_in{b}", [128, 512], BF16) for b in range(NBLK)]
    cc_out = [
        nc.dram_tensor(f"cc_out{b}", [GROUP_SIZE * 128, 512], BF16)
        for b in range(NBLK)
    ]

    with tile.TileContext(nc) as tc:
        with (
            tc.tile_pool(name="big", bufs=2) as big,  # xt, then the AG results
            tc.tile_pool(name="sb", bufs=1) as sb,
            tc.tile_pool(name="expp", bufs=4) as expp,
            tc.tile_pool(name="yout", bufs=3) as yout,
            tc.tile_pool(name="norm", bufs=8) as normp,
            tc.tile_pool(name="zv", bufs=4) as zvp,
            tc.tile_pool(name="psd", bufs=2, space="PSUM") as psd,  # 4 banks
            tc.tile_pool(name="pso", bufs=2, space="PSUM") as pso,  # 2 banks
            # shared 2-slot ring for Z-broadcast + out-proj accumulators
            tc.tile_pool(name="psx", bufs=2, space="PSUM") as psx,  # 2 banks
        ):
            # ---- load inputs -------------------------------------------------
            xt_sb = big.tile([128, KC, N], BF16, tag="bigbuf")
            wq_sb = sb.tile([128, KC, CS], BF16)
            wk_sb = sb.tile([128, KC, CS], BF16)
            wv_sb = sb.tile([128, KC, CS], BF16)
            wo_sb = sb.tile([128, KC, CS], BF16)
            nc.sync.dma_start(out=wq_sb, in_=wq.rearrange("(c p) n -> p c n", p=128))
            nc.sync.dma_start(out=wk_sb, in_=wk.rearrange("(c p) n -> p c n", p=128))
            xt_r = xt.rearrange("(c p) n -> p c n", p=128)
            for c in range(KC):
                for qb in range(QB):
                    sl = slice(qb * 512, (qb + 1) * 512)
                    nc.sync.dma_start(out=xt_sb[:, c, sl], in_=xt_r[:, c, sl])
            nc.sync.dma_start(out=wv_sb, in_=wv.rearrange("(c p) n -> p c n", p=128))
            nc.sync.dma_start(out=wo_sb, in_=wo.rearrange("(c p) n -> p c n", p=128))

            # bias, transposed layout: partition = column-within-block
            bias_sb = sb.tile([128, 2], F32)
            nc.sync.dma_start(out=bias_sb, in_=bo.rearrange("(cb p) -> p cb", p=128))

            ones_f = sb.tile([128, TT], F32)
            nc.vector.memset(ones_f, 1.0)
            ones_r = sb.tile([1, DH], F32R)
            nc.vector.tensor_copy(ones_r, ones_f[0:1, 0:1].broadcast_to([1, DH]))

            # ---- QKV projection ---------------------------------------------
            qt_sb = sb.tile([128, 2, N], BF16)
            kt_sb = sb.tile([128, 2, N], BF16)
            vaug = sb.tile([128, TT, HPC, DH + 1], BF16)
            with nc.allow_low_precision(reason="bf16 ones column"):
                for h in range(HPC):
                    nc.vector.tensor_copy(vaug[:, :, h, DH], ones_f)

            def qk_pass(m, dst, w_sb):
                # weight-stationary: one LDWEIGHTS per (m, c); 4 query blocks
                # stream through the same loaded weights.
                pa = psd.tile([128, 2, 512], F32, name="psd")
                pb = psd.tile([128, 2, 512], F32, name="psd")
                accs = [pa[:, 0, :], pa[:, 1, :], pb[:, 0, :], pb[:, 1, :]]
                for c in range(KC):
                    for qb in range(QB):
                        nc.tensor.matmul(
                            accs[qb],
                            lhsT=w_sb[:, c, m * 128 : (m + 1) * 128],
                            rhs=xt_sb[:, c, qb * 512 : (qb + 1) * 512],
                            start=(c == 0),
                            stop=(c == KC - 1),
                        )
                with nc.allow_low_precision(reason="bf16 attention"):
                    for qb in range(QB):
                        nc.vector.tensor_copy(
                            dst[:, m, qb * 512 : (qb + 1) * 512], accs[qb]
                        )

            for m in range(2):
                qk_pass(m, qt_sb, wq_sb)
                qk_pass(m, kt_sb, wk_sb)
            for t in range(TT):
                ps = psd.tile([128, 2, 512], F32, name="psd")
                acc = ps[:, 0, 0:CS]
                for c in range(KC):
                    nc.tensor.matmul(
                        acc,
                        lhsT=xt_sb[:, c, t * 128 : (t + 1) * 128],
                        rhs=wv_sb[:, c, :],
                        start=(c == 0),
                        stop=(c == KC - 1),
                    )
                with nc.allow_low_precision(reason="bf16 attention"):
                    nc.vector.tensor_copy(
                        vaug[:, t, :, 0:DH],
                        acc.rearrange("p (h d) -> p h d", d=DH),
                    )

            # ---- attention + per-block AllGather + interleaved out-proj -----
            outt_sb = sb.tile([128, 2, N], BF16)
            y_acc = sb.tile([128, 2, N], F32)
            ag_all = big.tile(
                [128, 2, QB, GROUP_SIZE, 512], BF16, tag="bigbuf"
            )  # [p, hp, qb, src_core, tok]

            def emit_dots(blk, kc):
                hp, qb = divmod(blk, QB)
                ps = psd.tile([128, 2, 512], F32, name="psd")
                for hh in range(2):
                    base = hh * DH
                    nc.tensor.matmul(
                        ps[:, hh, :],
                        lhsT=kt_sb[base : base + DH, hp, kc * 128 : (kc + 1) * 128],
                        rhs=qt_sb[base : base + DH, hp, qb * 512 : (qb + 1) * 512],
                        start=True,
                        stop=True,
                        tile_position=(base, 0),
                    )
                ex = expp.tile([128, 2, 512], BF16, name="expT")
                nc.scalar.activation(
                    out=ex, in_=ps, func=mybir.ActivationFunctionType.Exp, scale=SCALE
                )
                return ex

            def emit_attv(blk, kc, ex, po):
                hp = blk // QB
                for hh in range(2):
                    nc.tensor.matmul(
                        po[hh],
                        lhsT=vaug[:, kc, hp * 2 + hh, :],
                        rhs=ex[:, hh, :],
                        start=(kc == 0),
                        stop=(kc == NKC - 1),
                    )

            def emit_posb(po):
                # drain PSUM accumulators to SBUF right away to free the ring
                po_sbs = []
                for hh in range(2):
                    po_sb = normp.tile([DH + 1, 512], F32, name="po_sb")
                    nc.vector.tensor_copy(po_sb, po[hh])
                    po_sbs.append(po_sb)
                return po_sbs

            def emit_zrow(po_sbs):
                # raw Z as a rank-1 f32r row: the PE broadcast below must NOT
                # depend on the slow DVE reciprocal.
                zrows = []
                for hh in range(2):
                    zr = zvp.tile([1, 512], F32R, name="zr")
                    with nc.allow_low_precision(reason="f32r Z"):
                        nc.vector.tensor_copy(zr, po_sbs[hh][DH : DH + 1, :])
                    zrows.append(zr)
                return zrows

            def emit_zb(zrows, hh):
                # broadcast raw Z across 64 partitions (PE rank-1 matmul)
                zb = psx.tile([DH, 512], F32, name="zb", tag="psx")
                nc.tensor.matmul(zb, lhsT=ones_r, rhs=zrows[hh], start=True, stop=True)
                return zb

            def emit_recip(zb):
                zbi = normp.tile([DH, 512], F32, name="zbi")
                nc.vector.reciprocal(out=zbi, in_=zb)
                return zbi

            def emit_mul(blk, po_sbs, zbi, hh):
                hp, qb = divmod(blk, QB)
                base = hh * DH
                with nc.allow_low_precision(reason="bf16 attention out"):
                    nc.vector.tensor_mul(
                        outt_sb[base : base + DH, hp, qb * 512 : (qb + 1) * 512],
                        po_sbs[hh][0:DH, :],
                        zbi,
                    )

            def emit_ag(blk):
                hp, qb = divmod(blk, QB)
                sl = slice(qb * 512, (qb + 1) * 512)
                nc.gpsimd.dma_start(out=cc_in[blk].ap(), in_=outt_sb[:, hp, sl])
                nc.gpsimd.collective_compute(
                    "AllGather",
                    mybir.AluOpType.bypass,
                    ins=[cc_in[blk].ap().opt()],
                    outs=[cc_out[blk].ap().opt()],
                    replica_groups=REPLICA_GROUPS,
                )
                nc.sync.dma_start(
                    out=ag_all[:, hp, qb, :, :],
                    in_=cc_out[blk].ap().rearrange("(c p) n -> p c n", p=128),
                )

            # --- interleaved output projection -------------------------------
            # proj half-piece (hp, qb): y^T[:, qb] (+)= wo[hp]^T @ ag[hp][qb].
            # hp=0 initializes y_acc (bias folded in); hp=1 adds and stores.
            # Each half is 8 matmuls (2 col-blocks x 4 gathered cores) plus 2
            # DVE ops, broken into single-step chunks so it rides the PE slack
            # of the ACT-bound attention loop.
            def proj_tasks(hp, qb):
                tasks = []
                ps_ref = {}

                def mk_mm(cb, c0):
                    def f():
                        if c0 == 0:
                            ps_ref[cb] = psx.tile([128, 512], F32, name="psy", tag="psx")
                        for c in range(c0, c0 + 2):
                            nc.tensor.matmul(
                                ps_ref[cb],
                                lhsT=wo_sb[:, hp * 4 + c, cb * 128 : (cb + 1) * 128],
                                rhs=ag_all[:, hp, qb, c, :],
                                start=(c == 0),
                                stop=(c == 3),
                            )

                    return f

                def mk_fin(cb):
                    def f():
                        qsl = slice(qb * 512, (qb + 1) * 512)
                        if hp == 0:
                            nc.vector.tensor_scalar_add(
                                out=y_acc[:, cb, qsl],
                                in0=ps_ref[cb],
                                scalar1=bias_sb[:, cb : cb + 1],
                            )
                        else:
                            y_sb = yout.tile([128, 512], F32, name="y_sb")
                            nc.vector.tensor_add(y_sb, ps_ref[cb], y_acc[:, cb, qsl])
                            nc.sync.dma_start(
                                out=y[cb * 128 : (cb + 1) * 128, qsl], in_=y_sb
                            )

                    return f

                for cb in range(2):
                    tasks.append(mk_mm(cb, 0))
                    tasks.append(mk_mm(cb, 2))
                    tasks.append(mk_fin(cb))
                return tasks

            # schedule: during block b, emit proj for pieces whose AG fired
            # ~1.5 blocks earlier. AG(b) fires at (b, kc==7).
            proj_sched = {
                2: [(0, 0)],
                3: [(0, 1)],
                4: [(0, 2)],
                5: [(0, 3)],
                6: [(1, 0)],
                7: [(1, 1), (1, 2)],
            }

            # one continuous software-pipelined stream over all 8 blocks:
            # attV lags dots/exp by one step; po drains to SBUF right after a
            # block's last attV; recip/broadcast/mul stages are spread over
            # the next block's early steps; the AllGather fires at kc==7.
            pend_attv = None  # (blk, kc, ex)
            po_cur = None
            posb_prev = None  # po_sbs of previous block
            zrows_prev = None
            zb_prev = [None, None]
            zbi_prev = [None, None]
            task_q = []
            for step in range(NBLK * NKC):
                blk, kc = divmod(step, NKC)
                if kc == 0:
                    po_prev = po_cur
                    po_cur = [
                        pso.tile([DH + 1, 512], F32, name="ps_o") for _ in range(2)
                    ]
                    task_q = [
                        t for hq in proj_sched.get(blk, []) for t in proj_tasks(*hq)
                    ]
                ex = emit_dots(blk, kc)
                if pend_attv is not None:
                    pblk, pkc, pex = pend_attv
                    emit_attv(pblk, pkc, pex, po_cur if pblk == blk else po_prev)
                    if pkc == NKC - 1:
                        posb_cur = emit_posb(po_prev)
                pend_attv = (blk, kc, ex)
                if blk > 0:
                    if kc == 0:
                        posb_prev = posb_cur
                    elif kc == 1:
                        zrows_prev = emit_zrow(posb_prev)
                    elif kc == 2:
                        zb_prev[0] = emit_zb(zrows_prev, 0)
                    elif kc == 3:
                        zb_prev[1] = emit_zb(zrows_prev, 1)
                    elif kc == 4:
                        zbi_prev[0] = emit_recip(zb_prev[0])
                    elif kc == 5:
                        zbi_prev[1] = emit_recip(zb_prev[1])
                    elif kc == 6:
                        emit_mul(blk - 1, posb_prev, zbi_prev[0], 0)
                    elif kc == 7:
                        emit_mul(blk - 1, posb_prev, zbi_prev[1], 1)
                    elif kc == 8:
                        emit_ag(blk - 1)
                if kc >= 9 and task_q:
                    task_q.pop(0)()
            # drain: finish leftover proj tasks, then the last block's
            # norm (1/Z via exp(-ln Z) on the now-idle scalar engine), its
            # AllGather, and the final projection piece.
            for t in task_q:
                t()
            pblk, pkc, pex = pend_attv
            emit_attv(pblk, pkc, pex, po_cur)
            po_sbs = emit_posb(po_cur)
            for hh in range(2):
                zln = zvp.tile([1, 512], F32, name="zln")
                nc.scalar.activation(
                    out=zln,
                    in_=po_sbs[hh][DH : DH + 1, :],
                    func=mybir.ActivationFunctionType.Ln,
                )
                zinv = zvp.tile([1, 512], F32, name="zinv")
                nc.scalar.activation(
                    out=zinv,
                    in_=zln,
                    func=mybir.ActivationFunctionType.Exp,
                    scale=-1.0,
                )
                zir = zvp.tile([1, 512], F32R, name="zir")
                with nc.allow_low_precision(reason="f32r zinv"):
                    nc.vector.tensor_copy(zir, zinv)
                zb = psx.tile([DH, 512], F32, name="zb", tag="psx")
                nc.tensor.matmul(zb, lhsT=ones_r, rhs=zir, start=True, stop=True)
                hp, qb = divmod(NBLK - 1, QB)
                base = hh * DH
                with nc.allow_low_precision(reason="bf16 attention out"):
                    nc.vector.tensor_mul(
                        outt_sb[base : base + DH, hp, qb * 512 : (qb + 1) * 512],
                        po_sbs[hh][0:DH, :],
                        zb,
                    )
            emit_ag(NBLK - 1)
            for t in proj_tasks(1, QB - 1):
                t()

    nc.compile()
    return nc


_NC_CACHE = None


def _get_nc():
    global _NC_CACHE
    if _NC_CACHE is None:
        _NC_CACHE = build_nc()
    return _NC_CACHE


def _wo_perm(w_out):
    # chunk order [AG-hp0: r0..r3 -> w_out rows 256r..256r+128,
    #              AG-hp1: r0..r3 -> w_out rows 256r+128..256r+256]
    blocks = [w_out[256 * r : 256 * r + 128] for r in range(4)]
    blocks += [w_out[256 * r + 128 : 256 * r + 256] for r in range(4)]
    return np.concatenate(blocks, axis=0)


def _make_in_maps(x, w_qkv, w_out, b_out):
    wop = _wo_perm(w_out)
    in_maps = []
    for c in range(CORES):
        bi = c // GROUP_SIZE
        g = c % GROUP_SIZE
        cols = slice(g * CS, (g + 1) * CS)
        in_maps.append(
            {
                "xt": np.ascontiguousarray(x[bi].T).astype(NP_BF16),
                "wq": np.ascontiguousarray(w_qkv[:, cols]).astype(NP_BF16),
                "wk": np.ascontiguousarray(w_qkv[:, INNER:][:, cols]).astype(NP_BF16),
                "wv": np.ascontiguousarray(w_qkv[:, 2 * INNER:][:, cols]).astype(
                    NP_BF16
                ),
                "wo": np.ascontiguousarray(wop[:, cols]).astype(NP_BF16),
                "bo": np.ascontiguousarray(b_out[cols]),
            }
        )
    return in_maps


def _assemble(results):
    out = np.empty((B, N, DIM), dtype=np.float32)
    for c in range(CORES):
        bi = c // GROUP_SIZE
        g = c % GROUP_SIZE
        out[bi, :, g * CS : (g + 1) * CS] = results[c]["y"].T
    return out


def kernel(x, w_qkv, w_out, b_out, _trace=False, _trace_kwargs=None):
    x = np.asarray(x, dtype=np.float32)
    w_qkv = np.asarray(w_qkv, dtype=np.float32)
    w_out = np.asarray(w_out, dtype=np.float32)
    b_out = np.asarray(b_out, dtype=np.float32)
    nc = _get_nc()
    in_maps = _make_in_maps(x, w_qkv, w_out, b_out)
    res = run_bass_kernel_spmd(
        nc,
        in_maps,
        core_ids=list(range(CORES)),
        trace=_trace,
        **(_trace_kwargs or {}),
    )
    out = _assemble(res.results)
    if _trace:
        return out, res
    return out
